# revision 7
# baseline (speedup 1.0000x reference)
"""Trainium2 Bass kernel for ChebyshevAdditiveAngularMargin loss.

Reference computation (per element of a [N, C] f32 matrix):
    cosine = clip(outputs, -1+eps, 1-eps)
    phi    = clenshaw(cosine, coeffs)            # degree-30 Chebyshev
    phi    = where(cosine > TH, phi, cosine - MM)
    out    = SCALE * (targets * phi + (1 - targets) * cosine)

`targets` is a one-hot matrix (one 1.0 per row), so out == SCALE*cosine
everywhere except a single element per row.  The kernel therefore:
  1. extracts the hot cosine per row exactly via a fused multiply +
     row-sum (scalar_tensor_tensor accum_out; all non-hot products are
     exactly 0.0),
  2. runs the exact 31-step Clenshaw recurrence on just [128, 1]
     per-row values on the otherwise-idle scalar (ACT) engine — each op
     is y = scale*x + bias with scale in {1,-1,2,mask} or bias 0, i.e.
     exactly one fp32 rounding, reproducing jax's fp32 op order,
  3. scatters the correction back with one fused DVE op:
     out = (targets * delta[row]) + cosine, then * SCALE.

Per chunk of 128 rows the vector engine does one 1x extract, one 2x
clip, one 1x scatter and one 2x scale pass (~3 cycles/element total),
under the ~283us/core DMA floor (96 MB @ ~358 GB/s).  The extract's
mandatory full-size product output is dumped to PSUM to save SBUF for
deeper DMA double-buffering.

Rows are sharded across 8 NeuronCores (data parallel); the coefficient
vector is baked into the instruction stream as immediates.
"""

import sys

sys.path.insert(0, "/opt/trn_rl_repo")

import numpy as np

import concourse.bacc as bacc
import concourse.mybir as mybir
from concourse.tile import TileContext

F32 = mybir.dt.float32
OP = mybir.AluOpType
AF = mybir.ActivationFunctionType

N, C = 8192, 8192
N_CORES = 8
ROWS = N // N_CORES  # rows per core
P = 128  # SBUF partitions
PSUM_F = 4096  # PSUM free-dim capacity at f32 (2KB x 8 banks / 4B)

MARGIN = 0.2
SCALE = 30.0
EPS = 1e-07
TH = float(np.cos(np.pi - MARGIN))
MM = float(np.sin(np.pi - MARGIN) * MARGIN)
CLIP_LO = -1.0 + EPS
CLIP_HI = 1.0 - EPS


def build_bass(rows: int, cols: int, coeffs: np.ndarray):
    """Build the per-core program. Each core processes [rows, cols]."""
    cs = [float(c) for c in coeffs]  # f32 values, baked as immediates
    deg = len(cs) - 1
    n_blocks = rows // P

    nc = bacc.Bacc("TRN2", target_bir_lowering=False)
    x_d = nc.dram_tensor("outputs", [rows, cols], F32, kind="ExternalInput")
    t_d = nc.dram_tensor("targets", [rows, cols], F32, kind="ExternalInput")
    o_d = nc.dram_tensor("out", [rows, cols], F32, kind="ExternalOutput")

    def mad(out, in_, scale, bias):
        # out = fl(in_*scale + bias); with scale in {1.,-1.,2.,exact} or
        # bias==0. this is exactly one fp32 rounding
        nc.scalar.activation(out[:], in_[:], AF.Identity, bias=bias, scale=scale)

    def mad_c(out, c, bias_ap):
        # out = fl(c + bias); input is the registered const-1.0 tile so the
        # scale*in product is exactly c (only float biases 0/1 have consts)
        one = nc.const_aps.tensor(1.0, (P, 1))
        nc.scalar.activation(out[:], one, AF.Identity, bias=bias_ap[:], scale=c)

    with TileContext(nc) as tc:
        with (
            tc.tile_pool(name="xp", bufs=3) as xp,
            tc.tile_pool(name="tp", bufs=2) as tp,
            tc.tile_pool(name="ps", bufs=1, space="PSUM") as sp,
            tc.tile_pool(name="tiny", bufs=2) as yp,
        ):
            psum_f = min(PSUM_F, cols)
            scratch = sp.tile([P, psum_f], F32)  # extract's mandatory out
            n_half = cols // psum_f
            for b in range(n_blocks):
                xt = xp.tile([P, cols], F32, tag="xt")
                tt = tp.tile([P, cols], F32, tag="tt")
                nc.sync.dma_start(xt[:], x_d[b * P : (b + 1) * P, :])
                nc.sync.dma_start(tt[:], t_d[b * P : (b + 1) * P, :])

                # --- extract: s_raw[p] = sum_c targets*x (== hot x, exact)
                accs = []
                for h in range(n_half):
                    acc = yp.tile([P, 1], F32, tag=f"acc{h}")
                    sl = slice(h * psum_f, (h + 1) * psum_f)
                    nc.vector.scalar_tensor_tensor(
                        scratch[:], tt[:, sl], 1.0, xt[:, sl], OP.mult, OP.mult,
                        accum_out=acc[:],
                    )
                    accs.append(acc)
                if n_half > 1:
                    s_raw = yp.tile([P, 1], F32, tag="s_raw")
                    mad(s_raw, accs[0], 1.0, accs[1][:])  # exact: 1 term is 0.0
                else:
                    s_raw = accs[0]

                # --- big pass A (DVE 2x): cosine = clip(x), in place
                nc.vector.tensor_scalar(
                    xt[:], xt[:], CLIP_HI, CLIP_LO, OP.min, OP.max
                )

                # --- tiny path on ACT: clip, Clenshaw, select, delta ---
                s = yp.tile([P, 1], F32, tag="s")
                x2s = yp.tile([P, 1], F32, tag="x2s")
                nc.vector.tensor_scalar(  # clip (min+max) stays on DVE
                    s[:], s_raw[:], CLIP_HI, CLIP_LO, OP.min, OP.max
                )
                mad(x2s, s, 2.0, 0.0)  # exact *2

                b1 = yp.tile([P, 1], F32, tag="b1")
                b2 = yp.tile([P, 1], F32, tag="b2")
                bn = yp.tile([P, 1], F32, tag="bn")
                tm = yp.tile([P, 1], F32, tag="tm")
                tm2 = yp.tile([P, 1], F32, tag="tm2")
                nc.vector.memset(b1[:], cs[deg])  # step k=deg from (0,0)
                nc.vector.memset(b2[:], 0.0)
                for k in range(deg - 1, -1, -1):
                    # b_new = (c_k + x2*b1) - b2 rounded exactly like jax:
                    # tm = fl(x2*b1); tm2 = fl(tm + c_k); bn = fl(tm2 - b2)
                    mad(tm, b1, x2s[:], 0.0)
                    mad_c(tm2, cs[k], tm)
                    mad(bn, b2, -1.0, tm2[:])
                    b1, b2, bn = bn, b1, b2
                # phi = b0 - b1*x  (post-loop: b0 is b1, b1 is b2)
                mad(tm, b2, s[:], 0.0)
                phi = yp.tile([P, 1], F32, tag="phi")
                mad(phi, tm, -1.0, b1[:])

                # phisel = where(s > TH, phi, s - MM)
                mask = yp.tile([P, 1], F32, tag="mask")
                alt = yp.tile([P, 1], F32, tag="alt")
                diff = yp.tile([P, 1], F32, tag="diff")
                nc.vector.tensor_scalar(mask[:], s[:], TH, None, OP.is_gt)
                mad_c(alt, -MM, s)
                mad(diff, alt, -1.0, phi[:])
                phisel = yp.tile([P, 1], F32, tag="phisel")
                mad(phisel, diff, mask[:], alt[:])
                delta = yp.tile([P, 1], F32, tag="delta")
                mad(delta, s, -1.0, phisel[:])

                # --- big pass C (DVE 1x): out = (targets*delta[row]) + cosine
                nc.vector.scalar_tensor_tensor(
                    xt[:], tt[:], delta[:], xt[:], OP.mult, OP.add
                )
                # --- big pass D (DVE 2x): out *= SCALE
                nc.vector.tensor_scalar_mul(xt[:], xt[:], SCALE)

                nc.sync.dma_start(o_d[b * P : (b + 1) * P, :], xt[:])
    return nc


_TRACE = False  # test.py sets this to capture an NTFF profile
_LAST_RESULTS = None


def kernel(outputs: np.ndarray, targets: np.ndarray, coeffs: np.ndarray) -> np.ndarray:
    global _LAST_RESULTS
    from concourse.bass_utils import run_bass_kernel_spmd

    assert outputs.shape == (N, C) and targets.shape == (N, C)
    nc = build_bass(ROWS, C, np.asarray(coeffs))
    nc.finalize()
    in_maps = [
        {
            "outputs": np.ascontiguousarray(outputs[i * ROWS : (i + 1) * ROWS]),
            "targets": np.ascontiguousarray(targets[i * ROWS : (i + 1) * ROWS]),
        }
        for i in range(N_CORES)
    ]
    res = run_bass_kernel_spmd(
        nc, in_maps, core_ids=list(range(N_CORES)), trace=_TRACE
    )
    _LAST_RESULTS = res
    return np.concatenate([r["out"] for r in res.results], axis=0)
